# revision 13
# baseline (speedup 1.0000x reference)
"""Merged QKV linear + routed int4-LoRA delta on 8 Trainium2 NeuronCores.

Strategy (tensor-parallel along the QKV output dim, as in vLLM
ColumnParallelLinear): each core owns 768 output rows (512 q + 128 k + 128 v);
x is replicated. Tokens are sorted by adapter on the host so each contiguous
block uses ONE merged weight W + Wd[d] — the merged GEMM does base+delta in a
single pass (half the FLOPs). int4 delta weights stay packed in HBM and are
dequantized on-chip: DVE nibble unpack at int16 rate, then a fused
scalar_tensor_tensor (nib * sc + W') where W' is the base-weight shard with the
zero-point term folded in host-side. Matmuls are bf16 x bf16 -> fp32 PSUM.
"""
import numpy as np
import ml_dtypes

bf16 = ml_dtypes.bfloat16

D_ADAPTERS = 4
HIDDEN = 4096
Q_SIZE = 4096
KV_SIZE = 1024
TOKENS = 4096
PACK = 8
OUT = Q_SIZE + 2 * KV_SIZE
N_CORES = 8
FQ = Q_SIZE // N_CORES          # 512 q rows per core
FK = KV_SIZE // N_CORES         # 128 k (and v) rows per core
F = FQ + 2 * FK                 # 768 output rows per core
PQ = FQ // PACK                 # 64 packed q rows per core
PKV = FK // PACK                # 16 packed k/v rows per core
NPC = PQ + 2 * PKV              # 96 packed rows per core
HB = HIDDEN // 128              # 32 hidden tiles

_program_cache = {}


def _dev_perm():
    """dev col (j4*192 + 2p + h) -> local output row (8p + 4h + j4 per slice)."""
    perm = np.empty(F, np.int64)
    for j4 in range(4):
        for q in range(192):
            p, h = q // 2, q % 2
            if p < PQ:
                row = 8 * p + 4 * h + j4
            elif p < PQ + PKV:
                row = FQ + 8 * (p - PQ) + 4 * h + j4
            else:
                row = FQ + FK + 8 * (p - PQ - PKV) + 4 * h + j4
            perm[j4 * 192 + q] = row
    return perm


def _build_program(tile_adapter):
    import concourse.bacc as bacc
    import concourse.mybir as mybir
    import concourse.tile as tile

    nt = len(tile_adapter)
    nc = bacc.Bacc(None, target_bir_lowering=False)
    dt = mybir.dt

    xt = nc.dram_tensor("xt", [nt, 128, HIDDEN], dt.bfloat16, kind="ExternalInput")
    wt = nc.dram_tensor("wt", [D_ADAPTERS, HB, 128, F], dt.bfloat16, kind="ExternalInput")
    pk = nc.dram_tensor("pk", [D_ADAPTERS, 128, HB * NPC], dt.int32, kind="ExternalInput")
    scb = nc.dram_tensor("scb", [128, HB * 12], dt.float32, kind="ExternalInput")
    o = nc.dram_tensor("o", [nt, 128, F], dt.float32, kind="ExternalOutput")

    adapters = sorted(set(int(d) for d in tile_adapter))
    tiles_of = {d: [ti for ti, a in enumerate(tile_adapter) if a == d] for d in adapters}

    with tile.TileContext(nc) as tc:
        with (
            tc.tile_pool(name="wm_pool", bufs=2 * HB) as wm_pool,
            tc.tile_pool(name="x_pool", bufs=5) as x_pool,
            tc.tile_pool(name="wst_pool", bufs=8) as wst_pool,
            tc.tile_pool(name="pk_pool", bufs=8) as pk_pool,
            tc.tile_pool(name="nib_pool", bufs=4) as nib_pool,
            tc.tile_pool(name="scb_pool", bufs=1) as scb_pool,
            tc.tile_pool(name="stage_pool", bufs=2) as stage_pool,
            tc.tile_pool(name="psum_pool", bufs=4, space="PSUM") as psum_pool,
        ):
            # weight-stream DMAs ride the Scalar HWDGE ring; x/out DMAs ride the
            # Sync ring — separate FIFOs so slot-gated weight loads can't
            # head-of-line-block the x tiles the PE is waiting on.
            scb_t = scb_pool.tile([128, HB * 12], dt.float32)
            nc.sync.dma_start(out=scb_t[:], in_=scb[:])

            # PE warmup: dummy matmuls on a zeroed tile bridge the idle head so
            # the HAM clock gate is at 2.4 GHz when the first real MM issues
            warm = scb_pool.tile([128, 512], dt.bfloat16, tag="warm", name="warm_src")
            nc.gpsimd.memset(warm[:], 0)
            warm_ps = psum_pool.tile([128, F], dt.float32, tag="ps", name="warm_ps")
            for _r in range(20):
                nc.tensor.matmul(warm_ps[:, 0:512], lhsT=warm[:, 0:128],
                                 rhs=warm[:, 0:512], start=True, stop=True)

            def build_steps(d):
                """Generator: build merged weight for adapter d, one h-tile per
                step. First yield delivers the list of 32 wm tiles."""
                wms = [wm_pool.tile([128, F], dt.bfloat16, tag="wm", name=f"wm_{d}_{i}")
                       for i in range(HB)]
                yield wms
                # 4 separate chunk tiles: dep tracking is tile-granular, so
                # the first build step starts as soon as chunk 0 lands; chunks
                # 1-3 are emitted lazily so prefetched x tiles can slot in
                # between them in the sync DMA FIFO
                cw = HB * NPC // 4
                def chunk_dma(q):
                    pk_c = pk_pool.tile([128, cw], dt.int32, tag="pkc", name=f"pk_{d}_{q}")
                    nc.sync.dma_start(out=pk_c[:], in_=pk[d][:, q * cw:(q + 1) * cw])
                    return pk_c
                pk_ts = [chunk_dma(0)]
                yield None          # checkpoint: chunk 0 emitted
                for i in range(HB):
                    if i % 8 == 0 and i > 0:
                        pk_ts.append(chunk_dma(i // 8))
                    nib = nib_pool.tile([128, F], dt.int16)
                    pk16 = pk_ts[i // 8][:].bitcast(dt.int16)
                    ii = i % 8
                    src16 = pk16[:, ii * 2 * NPC:(ii + 1) * 2 * NPC]  # [128, 192]
                    for j4 in range(4):
                        nc.vector.tensor_scalar(
                            out=nib[:, j4 * 192:(j4 + 1) * 192],
                            in0=src16,
                            scalar1=4 * j4,
                            scalar2=0xF,
                            op0=mybir.AluOpType.logical_shift_right,
                            op1=mybir.AluOpType.bitwise_and,
                        )
                    wtt = wst_pool.tile([128, F], dt.bfloat16)
                    nc.scalar.dma_start(out=wtt[:], in_=wt[d, i])
                    nib3 = nib[:].rearrange("p (j c) -> p j c", j=4)
                    wm3 = wms[i][:].rearrange("p (j c) -> p j c", j=4)
                    wt3 = wtt[:].rearrange("p (j c) -> p j c", j=4)
                    for sl, c0, c1 in ((0, 0, 128), (1, 128, 160), (2, 160, 192)):
                        # wm = nib * sc + W'   (W' has -z*sc folded in)
                        nc.vector.scalar_tensor_tensor(
                            out=wm3[:, :, c0:c1],
                            in0=nib3[:, :, c0:c1],
                            scalar=scb_t[:, i * 12 + sl * 4 + d:i * 12 + sl * 4 + d + 1],
                            in1=wt3[:, :, c0:c1],
                            op0=mybir.AluOpType.mult,
                            op1=mybir.AluOpType.add,
                        )
                    yield None

            def x_load(ti):
                xtile = x_pool.tile([128, HIDDEN], dt.bfloat16, tag="xtile", name=f"x_{ti}")
                nc.sync.dma_start(out=xtile[:], in_=xt[ti])
                return xtile

            def gemm_tile(ti, wms, xtile=None):
                if xtile is None:
                    xtile = x_load(ti)
                x3 = xtile[:].rearrange("p (i t) -> p i t", i=HB)
                ps = psum_pool.tile([128, F], dt.float32)
                for i in range(HB):
                    nc.tensor.matmul(
                        ps[:, 0:512], lhsT=x3[:, i, :], rhs=wms[i][:, 0:512],
                        start=(i == 0), stop=(i == HB - 1),
                    )
                    nc.tensor.matmul(
                        ps[:, 512:F], lhsT=x3[:, i, :], rhs=wms[i][:, 512:F],
                        start=(i == 0), stop=(i == HB - 1),
                    )
                st = stage_pool.tile([128, F], dt.float32)
                nc.scalar.copy(out=st[:], in_=ps[:])
                nc.sync.dma_start(out=o[ti], in_=st[:])

            # emission: build(adapters[0]) fully, then for each adapter overlap
            # its GEMM tiles with the next adapter's build steps
            gen = build_steps(adapters[0])
            wm_cur = next(gen)
            next(gen)                       # emit pk chunk 0 DMA first
            xpre = {ti: x_load(ti) for ti in tiles_of[adapters[0]][:3]}
            for _ in gen:
                pass
            for k, d in enumerate(adapters):
                nxt = adapters[k + 1] if k + 1 < len(adapters) else None
                gen_next = build_steps(nxt) if nxt is not None else None
                wm_next = next(gen_next) if gen_next is not None else None
                tiles = tiles_of[d]
                per = 8   # front-load next build: done ~halfway through the era
                done = False
                for j, ti in enumerate(tiles):
                    gemm_tile(ti, wm_cur, xtile=xpre.get(ti) if k == 0 else None)
                    if gen_next is not None and not done:
                        for _ in range(per):
                            try:
                                next(gen_next)
                            except StopIteration:
                                done = True
                                break
                if gen_next is not None and not done:
                    for _ in gen_next:
                        pass
                wm_cur = wm_next
    nc.compile()
    return nc


def _prep(x, indices, W, qw_q, qw_k, qw_v, qz_q, qz_k, qz_v, sc_q, sc_k, sc_v):
    """Host-side shard + layout prep. Returns (tile_adapter, in_maps, assemble_info)."""
    order = np.argsort(indices, kind="stable")
    counts = np.bincount(indices, minlength=D_ADAPTERS)
    nb = [int(-(-int(c) // 128)) for c in counts]
    nt = sum(nb)
    T_pad = 128 * nt

    tile_adapter = []
    x_sorted = np.zeros((T_pad, HIDDEN), np.float32)
    valid_rows = np.empty(TOKENS, np.int64)
    token_ids = np.empty(TOKENS, np.int64)
    row0 = 0
    t0 = 0
    n_valid = 0
    for d in range(D_ADAPTERS):
        cd = int(counts[d])
        if cd == 0:
            continue
        toks = order[t0:t0 + cd]
        x_sorted[row0:row0 + cd] = x[toks]
        valid_rows[n_valid:n_valid + cd] = np.arange(row0, row0 + cd)
        token_ids[n_valid:n_valid + cd] = toks
        tile_adapter.extend([d] * nb[d])
        n_valid += cd
        row0 += 128 * nb[d]
        t0 += cd

    # x tiles: [nt, 128p, (hb t)] with A[ti, p, hb*128+t] = x_sorted[ti*128+t, hb*128+p]
    xtiles = np.ascontiguousarray(
        x_sorted.astype(bf16).reshape(nt, 128, HB, 128).transpose(0, 3, 2, 1).reshape(nt, 128, HIDDEN)
    )

    perm = _dev_perm()
    shifts = np.arange(PACK, dtype=np.uint32) * 4

    def unpack_z(qz):
        return ((qz.astype(np.uint32)[:, :, None] >> shifts[None, None, :]) & 0xF).reshape(
            D_ADAPTERS, HIDDEN).astype(np.float32)

    z_all = [unpack_z(qz_q), unpack_z(qz_k), unpack_z(qz_v)]
    sc_all = [np.asarray(s, np.float32) for s in (sc_q, sc_k, sc_v)]

    # scb: [128, hb*12]: col hb*12 + sl*4 + d = sc_sl[d, hb*128+p]
    scb = np.empty((128, HB, 12), np.float32)
    for sl in range(3):
        for d in range(D_ADAPTERS):
            scb[:, :, sl * 4 + d] = sc_all[sl][d].reshape(HB, 128).T
    scb = np.ascontiguousarray(scb.reshape(128, HB * 12))

    # per-column slice id and -z*sc bias per (adapter, h, devcol)
    slice_of_col = np.empty(F, np.int64)
    for j4 in range(4):
        slice_of_col[j4 * 192:j4 * 192 + 128] = 0
        slice_of_col[j4 * 192 + 128:j4 * 192 + 160] = 1
        slice_of_col[j4 * 192 + 160:j4 * 192 + 192] = 2

    in_maps = []
    for c in range(N_CORES):
        W_shard = np.concatenate([
            W[FQ * c:FQ * (c + 1)],
            W[Q_SIZE + FK * c:Q_SIZE + FK * (c + 1)],
            W[Q_SIZE + KV_SIZE + FK * c:Q_SIZE + KV_SIZE + FK * (c + 1)],
        ], 0)  # [768, H] local rows
        W_dev = W_shard[perm].T.astype(np.float32)          # [H, 768] dev cols
        # fold -z*sc into the base weight, per adapter
        wt_c = np.empty((D_ADAPTERS, HIDDEN, F), np.float32)
        for d in range(D_ADAPTERS):
            zs = np.stack([z_all[sl][d] * sc_all[sl][d] for sl in range(3)], 0)  # [3, H]
            wt_c[d] = W_dev - zs[slice_of_col, :].T
        wt_c = np.ascontiguousarray(
            wt_c.astype(bf16).reshape(D_ADAPTERS, HB, 128, F)
        )
        qw_cat = np.concatenate([
            qw_q[:, PQ * c:PQ * (c + 1), :],
            qw_k[:, PKV * c:PKV * (c + 1), :],
            qw_v[:, PKV * c:PKV * (c + 1), :],
        ], 1)  # [D, 96, H]
        pk_c = np.ascontiguousarray(
            qw_cat.transpose(0, 2, 1).reshape(D_ADAPTERS, HB, 128, NPC)
            .transpose(0, 2, 1, 3).reshape(D_ADAPTERS, 128, HB * NPC)
        )
        in_maps.append({"xt": xtiles, "wt": wt_c, "pk": pk_c, "scb": scb})

    info = (perm, valid_rows[:n_valid], token_ids[:n_valid], T_pad)
    return tuple(tile_adapter), in_maps, info


def _assemble(results, info):
    perm, valid_rows, token_ids, T_pad = info
    out = np.empty((TOKENS, OUT), np.float32)
    inv = np.empty(F, np.int64)
    inv[perm] = np.arange(F)
    for c in range(N_CORES):
        od = results[c]["o"].reshape(T_pad, F)
        loc = od[:, inv][valid_rows]          # [n_valid, 768] local row order
        out[token_ids, FQ * c:FQ * (c + 1)] = loc[:, 0:FQ]
        out[token_ids, Q_SIZE + FK * c:Q_SIZE + FK * (c + 1)] = loc[:, FQ:FQ + FK]
        out[token_ids, Q_SIZE + KV_SIZE + FK * c:Q_SIZE + KV_SIZE + FK * (c + 1)] = loc[:, FQ + FK:F]
    return out


def run(trace=False, **inputs):
    from concourse.bass_utils import run_bass_kernel_spmd

    args = {k: np.asarray(v) for k, v in inputs.items()}
    tile_adapter, in_maps, info = _prep(**args)
    if tile_adapter not in _program_cache:
        _program_cache[tile_adapter] = _build_program(tile_adapter)
    nc = _program_cache[tile_adapter]
    res = run_bass_kernel_spmd(nc, in_maps, core_ids=list(range(N_CORES)), trace=trace)
    out = _assemble(res.results, info)
    return out, res.exec_time_ns


def kernel(**inputs):
    out, _ = run(trace=False, **inputs)
    return out


# revision 14
# speedup vs baseline: 1.1803x; 1.1803x over previous
"""Merged QKV linear + routed int4-LoRA delta on 8 Trainium2 NeuronCores.

Strategy (tensor-parallel along the QKV output dim, as in vLLM
ColumnParallelLinear): each core owns 768 output rows (512 q + 128 k + 128 v);
x is replicated. Tokens are sorted by adapter on the host so each contiguous
block uses ONE merged weight W + Wd[d] — the merged GEMM does base+delta in a
single pass (half the FLOPs). int4 delta weights stay packed in HBM and are
dequantized on-chip: DVE nibble unpack at int16 rate, then a fused
scalar_tensor_tensor (nib * sc + W') where W' is the base-weight shard with the
zero-point term folded in host-side. Matmuls are bf16 x bf16 -> fp32 PSUM.
"""
import numpy as np
import ml_dtypes

bf16 = ml_dtypes.bfloat16

D_ADAPTERS = 4
HIDDEN = 4096
Q_SIZE = 4096
KV_SIZE = 1024
TOKENS = 4096
PACK = 8
OUT = Q_SIZE + 2 * KV_SIZE
N_CORES = 8
FQ = Q_SIZE // N_CORES          # 512 q rows per core
FK = KV_SIZE // N_CORES         # 128 k (and v) rows per core
F = FQ + 2 * FK                 # 768 output rows per core
PQ = FQ // PACK                 # 64 packed q rows per core
PKV = FK // PACK                # 16 packed k/v rows per core
NPC = PQ + 2 * PKV              # 96 packed rows per core
HB = HIDDEN // 128              # 32 hidden tiles

_program_cache = {}


def _dev_perm():
    """dev col (j4*192 + 2p + h) -> local output row (8p + 4h + j4 per slice)."""
    perm = np.empty(F, np.int64)
    for j4 in range(4):
        for q in range(192):
            p, h = q // 2, q % 2
            if p < PQ:
                row = 8 * p + 4 * h + j4
            elif p < PQ + PKV:
                row = FQ + 8 * (p - PQ) + 4 * h + j4
            else:
                row = FQ + FK + 8 * (p - PQ - PKV) + 4 * h + j4
            perm[j4 * 192 + q] = row
    return perm


def _build_program(tile_adapter):
    import concourse.bacc as bacc
    import concourse.mybir as mybir
    import concourse.tile as tile

    nt = len(tile_adapter)
    nc = bacc.Bacc(None, target_bir_lowering=False)
    dt = mybir.dt

    xt = nc.dram_tensor("xt", [nt, 128, HIDDEN], dt.bfloat16, kind="ExternalInput")
    wt = nc.dram_tensor("wt", [D_ADAPTERS, HB, 128, F], dt.bfloat16, kind="ExternalInput")
    pk = nc.dram_tensor("pk", [D_ADAPTERS, 128, HB * NPC], dt.int32, kind="ExternalInput")
    scb = nc.dram_tensor("scb", [128, HB * 12], dt.float32, kind="ExternalInput")
    o = nc.dram_tensor("o", [nt, 128, F], dt.float32, kind="ExternalOutput")

    adapters = sorted(set(int(d) for d in tile_adapter))
    tiles_of = {d: [ti for ti, a in enumerate(tile_adapter) if a == d] for d in adapters}

    with tile.TileContext(nc) as tc:
        with (
            tc.tile_pool(name="wm_pool", bufs=2 * HB) as wm_pool,
            tc.tile_pool(name="x_pool", bufs=5) as x_pool,
            tc.tile_pool(name="wst_pool", bufs=8) as wst_pool,
            tc.tile_pool(name="pk_pool", bufs=8) as pk_pool,
            tc.tile_pool(name="nib_pool", bufs=4) as nib_pool,
            tc.tile_pool(name="scb_pool", bufs=1) as scb_pool,
            tc.tile_pool(name="stage_pool", bufs=2) as stage_pool,
            tc.tile_pool(name="psum_pool", bufs=4, space="PSUM") as psum_pool,
        ):
            # weight-stream DMAs ride the Scalar HWDGE ring; x/out DMAs ride the
            # Sync ring — separate FIFOs so slot-gated weight loads can't
            # head-of-line-block the x tiles the PE is waiting on.
            scb_t = scb_pool.tile([128, HB * 12], dt.float32)
            nc.sync.dma_start(out=scb_t[:], in_=scb[:])

            def build_steps(d):
                """Generator: build merged weight for adapter d, one h-tile per
                step. First yield delivers the list of 32 wm tiles."""
                wms = [wm_pool.tile([128, F], dt.bfloat16, tag="wm", name=f"wm_{d}_{i}")
                       for i in range(HB)]
                yield wms
                # 4 separate chunk tiles: dep tracking is tile-granular, so
                # the first build step starts as soon as chunk 0 lands; chunks
                # 1-3 are emitted lazily so prefetched x tiles can slot in
                # between them in the sync DMA FIFO
                cw = HB * NPC // 4
                def chunk_dma(q):
                    pk_c = pk_pool.tile([128, cw], dt.int32, tag="pkc", name=f"pk_{d}_{q}")
                    nc.sync.dma_start(out=pk_c[:], in_=pk[d][:, q * cw:(q + 1) * cw])
                    return pk_c
                pk_ts = [chunk_dma(0)]
                yield None          # checkpoint: chunk 0 emitted
                for i in range(HB):
                    if i % 8 == 0 and i > 0:
                        pk_ts.append(chunk_dma(i // 8))
                    nib = nib_pool.tile([128, F], dt.int16)
                    pk16 = pk_ts[i // 8][:].bitcast(dt.int16)
                    ii = i % 8
                    src16 = pk16[:, ii * 2 * NPC:(ii + 1) * 2 * NPC]  # [128, 192]
                    for j4 in range(4):
                        nc.vector.tensor_scalar(
                            out=nib[:, j4 * 192:(j4 + 1) * 192],
                            in0=src16,
                            scalar1=4 * j4,
                            scalar2=0xF,
                            op0=mybir.AluOpType.logical_shift_right,
                            op1=mybir.AluOpType.bitwise_and,
                        )
                    wtt = wst_pool.tile([128, F], dt.bfloat16)
                    nc.scalar.dma_start(out=wtt[:], in_=wt[d, i])
                    nib3 = nib[:].rearrange("p (j c) -> p j c", j=4)
                    wm3 = wms[i][:].rearrange("p (j c) -> p j c", j=4)
                    wt3 = wtt[:].rearrange("p (j c) -> p j c", j=4)
                    for sl, c0, c1 in ((0, 0, 128), (1, 128, 160), (2, 160, 192)):
                        # wm = nib * sc + W'   (W' has -z*sc folded in)
                        nc.vector.scalar_tensor_tensor(
                            out=wm3[:, :, c0:c1],
                            in0=nib3[:, :, c0:c1],
                            scalar=scb_t[:, i * 12 + sl * 4 + d:i * 12 + sl * 4 + d + 1],
                            in1=wt3[:, :, c0:c1],
                            op0=mybir.AluOpType.mult,
                            op1=mybir.AluOpType.add,
                        )
                    yield None

            def x_load(ti):
                xtile = x_pool.tile([128, HIDDEN], dt.bfloat16, tag="xtile", name=f"x_{ti}")
                nc.sync.dma_start(out=xtile[:], in_=xt[ti])
                return xtile

            def gemm_tile(ti, wms, xtile=None):
                if xtile is None:
                    xtile = x_load(ti)
                x3 = xtile[:].rearrange("p (i t) -> p i t", i=HB)
                ps = psum_pool.tile([128, F], dt.float32)
                for i in range(HB):
                    nc.tensor.matmul(
                        ps[:, 0:512], lhsT=x3[:, i, :], rhs=wms[i][:, 0:512],
                        start=(i == 0), stop=(i == HB - 1),
                    )
                    nc.tensor.matmul(
                        ps[:, 512:F], lhsT=x3[:, i, :], rhs=wms[i][:, 512:F],
                        start=(i == 0), stop=(i == HB - 1),
                    )
                st = stage_pool.tile([128, F], dt.float32)
                nc.scalar.copy(out=st[:], in_=ps[:])
                nc.sync.dma_start(out=o[ti], in_=st[:])

            # emission: build(adapters[0]) fully, then for each adapter overlap
            # its GEMM tiles with the next adapter's build steps
            gen = build_steps(adapters[0])
            wm_cur = next(gen)
            next(gen)                       # emit pk chunk 0 DMA first
            xpre = {ti: x_load(ti) for ti in tiles_of[adapters[0]][:3]}
            for _ in gen:
                pass
            for k, d in enumerate(adapters):
                nxt = adapters[k + 1] if k + 1 < len(adapters) else None
                gen_next = build_steps(nxt) if nxt is not None else None
                wm_next = next(gen_next) if gen_next is not None else None
                tiles = tiles_of[d]
                per = 8   # front-load next build: done ~halfway through the era
                done = False
                for j, ti in enumerate(tiles):
                    gemm_tile(ti, wm_cur, xtile=xpre.get(ti) if k == 0 else None)
                    if gen_next is not None and not done:
                        for _ in range(per):
                            try:
                                next(gen_next)
                            except StopIteration:
                                done = True
                                break
                if gen_next is not None and not done:
                    for _ in gen_next:
                        pass
                wm_cur = wm_next
    nc.compile()
    return nc


def _prep(x, indices, W, qw_q, qw_k, qw_v, qz_q, qz_k, qz_v, sc_q, sc_k, sc_v):
    """Host-side shard + layout prep. Returns (tile_adapter, in_maps, assemble_info)."""
    order = np.argsort(indices, kind="stable")
    counts = np.bincount(indices, minlength=D_ADAPTERS)
    nb = [int(-(-int(c) // 128)) for c in counts]
    nt = sum(nb)
    T_pad = 128 * nt

    tile_adapter = []
    x_sorted = np.zeros((T_pad, HIDDEN), np.float32)
    valid_rows = np.empty(TOKENS, np.int64)
    token_ids = np.empty(TOKENS, np.int64)
    row0 = 0
    t0 = 0
    n_valid = 0
    for d in range(D_ADAPTERS):
        cd = int(counts[d])
        if cd == 0:
            continue
        toks = order[t0:t0 + cd]
        x_sorted[row0:row0 + cd] = x[toks]
        valid_rows[n_valid:n_valid + cd] = np.arange(row0, row0 + cd)
        token_ids[n_valid:n_valid + cd] = toks
        tile_adapter.extend([d] * nb[d])
        n_valid += cd
        row0 += 128 * nb[d]
        t0 += cd

    # x tiles: [nt, 128p, (hb t)] with A[ti, p, hb*128+t] = x_sorted[ti*128+t, hb*128+p]
    xtiles = np.ascontiguousarray(
        x_sorted.astype(bf16).reshape(nt, 128, HB, 128).transpose(0, 3, 2, 1).reshape(nt, 128, HIDDEN)
    )

    perm = _dev_perm()
    shifts = np.arange(PACK, dtype=np.uint32) * 4

    def unpack_z(qz):
        return ((qz.astype(np.uint32)[:, :, None] >> shifts[None, None, :]) & 0xF).reshape(
            D_ADAPTERS, HIDDEN).astype(np.float32)

    z_all = [unpack_z(qz_q), unpack_z(qz_k), unpack_z(qz_v)]
    sc_all = [np.asarray(s, np.float32) for s in (sc_q, sc_k, sc_v)]

    # scb: [128, hb*12]: col hb*12 + sl*4 + d = sc_sl[d, hb*128+p]
    scb = np.empty((128, HB, 12), np.float32)
    for sl in range(3):
        for d in range(D_ADAPTERS):
            scb[:, :, sl * 4 + d] = sc_all[sl][d].reshape(HB, 128).T
    scb = np.ascontiguousarray(scb.reshape(128, HB * 12))

    # per-column slice id and -z*sc bias per (adapter, h, devcol)
    slice_of_col = np.empty(F, np.int64)
    for j4 in range(4):
        slice_of_col[j4 * 192:j4 * 192 + 128] = 0
        slice_of_col[j4 * 192 + 128:j4 * 192 + 160] = 1
        slice_of_col[j4 * 192 + 160:j4 * 192 + 192] = 2

    in_maps = []
    for c in range(N_CORES):
        W_shard = np.concatenate([
            W[FQ * c:FQ * (c + 1)],
            W[Q_SIZE + FK * c:Q_SIZE + FK * (c + 1)],
            W[Q_SIZE + KV_SIZE + FK * c:Q_SIZE + KV_SIZE + FK * (c + 1)],
        ], 0)  # [768, H] local rows
        W_dev = W_shard[perm].T.astype(np.float32)          # [H, 768] dev cols
        # fold -z*sc into the base weight, per adapter
        wt_c = np.empty((D_ADAPTERS, HIDDEN, F), np.float32)
        for d in range(D_ADAPTERS):
            zs = np.stack([z_all[sl][d] * sc_all[sl][d] for sl in range(3)], 0)  # [3, H]
            wt_c[d] = W_dev - zs[slice_of_col, :].T
        wt_c = np.ascontiguousarray(
            wt_c.astype(bf16).reshape(D_ADAPTERS, HB, 128, F)
        )
        qw_cat = np.concatenate([
            qw_q[:, PQ * c:PQ * (c + 1), :],
            qw_k[:, PKV * c:PKV * (c + 1), :],
            qw_v[:, PKV * c:PKV * (c + 1), :],
        ], 1)  # [D, 96, H]
        pk_c = np.ascontiguousarray(
            qw_cat.transpose(0, 2, 1).reshape(D_ADAPTERS, HB, 128, NPC)
            .transpose(0, 2, 1, 3).reshape(D_ADAPTERS, 128, HB * NPC)
        )
        in_maps.append({"xt": xtiles, "wt": wt_c, "pk": pk_c, "scb": scb})

    info = (perm, valid_rows[:n_valid], token_ids[:n_valid], T_pad)
    return tuple(tile_adapter), in_maps, info


def _assemble(results, info):
    perm, valid_rows, token_ids, T_pad = info
    out = np.empty((TOKENS, OUT), np.float32)
    inv = np.empty(F, np.int64)
    inv[perm] = np.arange(F)
    for c in range(N_CORES):
        od = results[c]["o"].reshape(T_pad, F)
        loc = od[:, inv][valid_rows]          # [n_valid, 768] local row order
        out[token_ids, FQ * c:FQ * (c + 1)] = loc[:, 0:FQ]
        out[token_ids, Q_SIZE + FK * c:Q_SIZE + FK * (c + 1)] = loc[:, FQ:FQ + FK]
        out[token_ids, Q_SIZE + KV_SIZE + FK * c:Q_SIZE + KV_SIZE + FK * (c + 1)] = loc[:, FQ + FK:F]
    return out


def run(trace=False, **inputs):
    from concourse.bass_utils import run_bass_kernel_spmd

    args = {k: np.asarray(v) for k, v in inputs.items()}
    tile_adapter, in_maps, info = _prep(**args)
    if tile_adapter not in _program_cache:
        _program_cache[tile_adapter] = _build_program(tile_adapter)
    nc = _program_cache[tile_adapter]
    res = run_bass_kernel_spmd(nc, in_maps, core_ids=list(range(N_CORES)), trace=trace)
    out = _assemble(res.results, info)
    return out, res.exec_time_ns


def kernel(**inputs):
    out, _ = run(trace=False, **inputs)
    return out
